# revision 2
# baseline (speedup 1.0000x reference)
"""BitNet binary linear with mixed fp16 / fp8-DoubleRow k-chunks.

Same structure as the baseline kernel (tensor-parallel on out_features,
x replicated, host-packed SBUF layouts, f32 PSUM + bias at eviction),
but the 32 k-chunks are split:
  - first NH chunks: x in fp16, regular matmul (bf16-rate, exact-ish)
  - last N_DR chunks: x in fp8e4m3, DoubleRow matmul (2 chunks/MM at
    ~2x rate); s is ternary so it stays exact in fp8.
Quantization error comes only from the fp8 chunks:
rel_err ~= sqrt(N_DR/32) * 2.66e-2.
"""

import os
import numpy as np

B, S, D_IN, D_OUT = 4, 2048, 4096, 16384
N_CORES = 8
R = B * S                 # 8192 rows of x
F = D_OUT // N_CORES      # 2048 features per core
KC = D_IN // 128          # 32 k-chunks
RB = 512                  # steady-state r-block
FT = 512                  # f-tile (one PSUM bank)
NF = F // FT              # 4 f-tiles == wt quarters
NB = (R - 512) // RB      # 15 steady blocks (rows 512..8192)

N_DR = 16                                  # chunks on the DoubleRow path
assert N_DR % 2 == 0
NH = KC - N_DR                             # fp16 chunks
NPAIR = N_DR // 2

_CACHE = {}
_GAMMA = [np.float32(1.0)]


def _patch_light_exit():
    """Drop the second all-engine barrier in TileContext's exit (see
    baseline kernel for rationale)."""
    import concourse.tile as tile
    from concourse.vector_clock import ScopedClock

    if getattr(tile.TileContext, "_light_exit", False):
        return

    def _drain_and_barrier(self, tick_clock, wait_clock):
        nc = self.nc
        drain_inst = nc.sync.drain()
        wait_clock.add_sem_waits(
            drain_inst.ins, ScopedClock({None: tick_clock.global_clock})
        )
        nc.all_engine_barrier()
        popped = nc._tile_sem_poison_stack.pop()
        assert popped is self._sem_poison
        nc.clear_and_free_semaphores(list(self.sems.allocated().values()))

    tile.TileContext._drain_and_barrier = _drain_and_barrier
    tile.TileContext._light_exit = True


def _build_nc():
    import concourse.mybir as mybir
    import concourse.tile as tile
    from concourse import bacc

    _patch_light_exit()
    fp16 = mybir.dt.float16
    fp8 = mybir.dt.float8e4
    f32 = mybir.dt.float32
    DR = mybir.MatmulPerfMode.DoubleRow

    nc = bacc.Bacc("TRN2", target_bir_lowering=False, debug=False,
                   num_devices=N_CORES)
    # fp16 x (first NH chunks), packed [part, chunk, rows]
    if NH:
        xh0_16 = nc.declare_dram_parameter("xh0_16", [128, NH * 128], fp16, isOutput=False)
        xh1_16 = nc.declare_dram_parameter("xh1_16", [128, NH * 384], fp16, isOutput=False)
        xp16 = nc.declare_dram_parameter("xp16", [NB, 128, NH * RB], fp16, isOutput=False)
    # fp8 x (last N_DR chunks), packed [part, chunk, rows]
    if N_DR:
        xh0_8 = nc.declare_dram_parameter("xh0_8", [128, N_DR * 128], fp8, isOutput=False)
        xh1_8 = nc.declare_dram_parameter("xh1_8", [128, N_DR * 384], fp8, isOutput=False)
        xp8 = nc.declare_dram_parameter("xp8", [NB, 128, N_DR * RB], fp8, isOutput=False)
    wq = nc.declare_dram_parameter("wq", [NF, 128, KC * FT], fp8, isOutput=False)
    bias = nc.declare_dram_parameter("bias", [1, F], fp16, isOutput=False)
    out = nc.declare_dram_parameter("out", [R, F], f32, isOutput=True)

    with tile.TileContext(nc) as tc:
        with (
            tc.tile_pool(name="wpool", bufs=1) as wpool,
            tc.tile_pool(name="cpool", bufs=1) as cpool,
            tc.tile_pool(name="xpool", bufs=2) as xpool,
            tc.tile_pool(name="opool", bufs=4) as opool,
            tc.tile_pool(name="pspool", bufs=4, space="PSUM") as pspool,
        ):
            # broadcast bias across partitions once: ones[1,128].T @ bias[1,512]
            bias_sb = cpool.tile([1, F], fp16, tag="bias")
            nc.sync.dma_start(bias_sb[:], bias[:, :])
            ones_sb = cpool.tile([1, 128], fp16, tag="ones")
            nc.gpsimd.memset(ones_sb[:], 1.0)
            bias_bc = cpool.tile([128, F], f32, tag="bias_bc")
            for f in range(NF):
                bp = pspool.tile([128, FT], f32)
                nc.tensor.matmul(bp[:], ones_sb[:],
                                 bias_sb[:, f * FT:(f + 1) * FT],
                                 start=True, stop=True)
                nc.vector.tensor_copy(bias_bc[:, f * FT:(f + 1) * FT], bp[:])

            # head DMAs in critical-path order: first x rows, then wt
            # quarters (second x block slotted after the first quarter)
            xh0_16t = xh1_16t = xh0_8t = xh1_8t = None
            if NH:
                xh0_16t = cpool.tile([128, NH * 128], fp16, tag="xh0_16")
                nc.sync.dma_start(xh0_16t[:], xh0_16[:, :])
            if N_DR:
                xh0_8t = cpool.tile([128, N_DR * 128], fp8, tag="xh0_8")
                nc.sync.dma_start(xh0_8t[:], xh0_8[:, :])
            wt_sb = []
            for q in range(NF):
                t = wpool.tile([128, KC * FT], fp8, tag=f"wq{q}")
                nc.sync.dma_start(t[:], wq[q, :, :])
                wt_sb.append(t)
                if q == 0:
                    if NH:
                        xh1_16t = cpool.tile([128, NH * 384], fp16, tag="xh1_16")
                        nc.sync.dma_start(xh1_16t[:], xh1_16[:, :])
                    if N_DR:
                        xh1_8t = cpool.tile([128, N_DR * 384], fp8, tag="xh1_8")
                        nc.sync.dma_start(xh1_8t[:], xh1_8[:, :])

            def do_tile(x16_t, x8_t, rbn, rt, r0, f):
                ps = pspool.tile([128, FT], f32)
                for c in range(NH):
                    nc.tensor.matmul(
                        ps[:],
                        x16_t[:, c * rbn + rt * 128:c * rbn + rt * 128 + 128],
                        wt_sb[f][:, c * FT:(c + 1) * FT],
                        start=(c == 0), stop=False,
                    )
                for p in range(NPAIR):
                    lhs3 = x8_t[:, 2 * p * rbn:2 * (p + 1) * rbn].rearrange(
                        "q (two r) -> q two r", two=2
                    )[:, :, rt * 128:rt * 128 + 128]
                    gc = NH + 2 * p   # global chunk index of the pair
                    rhs3 = wt_sb[f][:, gc * FT:(gc + 2) * FT].rearrange(
                        "q (two n) -> q two n", two=2
                    )
                    nc.tensor.matmul(
                        ps[:], lhs3, rhs3,
                        start=(NH == 0 and p == 0), stop=(p == NPAIR - 1),
                        perf_mode=DR,
                    )
                ob = opool.tile([128, FT], f32)
                nc.vector.tensor_add(
                    ob[:], ps[:], bias_bc[:, f * FT:(f + 1) * FT]
                )
                nc.sync.dma_start(
                    out[r0:r0 + 128, f * FT:(f + 1) * FT], ob[:]
                )

            # prime: rows 0..512, one f-quarter at a time (PE is in-order;
            # quarter f+1 streams in while quarter f computes)
            for f in range(NF):
                do_tile(xh0_16t, xh0_8t, 128, 0, 0, f)
                for rt in range(3):
                    do_tile(xh1_16t, xh1_8t, 384, rt, 128 + rt * 128, f)

            # steady state
            for b in range(NB):
                x16_t = x8_t = None
                if NH:
                    x16_t = xpool.tile([128, NH * RB], fp16)
                    nc.sync.dma_start(x16_t[:], xp16[b, :, :])
                if N_DR:
                    x8_t = xpool.tile([128, N_DR * RB], fp8)
                    nc.sync.dma_start(x8_t[:], xp8[b, :, :])
                for rt in range(RB // 128):
                    for f in range(NF):
                        do_tile(x16_t, x8_t, RB, rt, 512 + b * RB + rt * 128, f)
    nc.compile()
    return nc


def _pack(a):
    """[rows, D_IN] -> [128, KC, rows] in SBUF layout (partition = k%128)."""
    rows = a.shape[0]
    return np.ascontiguousarray(a.T.reshape(KC, 128, rows).transpose(1, 0, 2))


def _prepare_in_maps(x, weight, bias):
    import ml_dtypes

    x = np.asarray(x)
    weight = np.asarray(weight)
    bias = np.asarray(bias)

    gamma = np.float32(max(np.mean(np.abs(weight), dtype=np.float64), 1e-5))
    s = np.clip(np.rint(weight.astype(np.float32) / gamma), -1.0, 1.0)
    _GAMMA[0] = gamma

    # x is quantized UNSCALED (sigma ~ 1 sits in e4m3's sweet spot; folding
    # gamma ~ 2^-7 in would push values into fp8 subnormals). The device
    # computes x@s.T + bias/gamma; the host multiplies by gamma after.
    xs = x.reshape(R, D_IN)

    def pack_both(rows_lo, rows_hi):
        p = _pack(xs[rows_lo:rows_hi])          # [128, KC, rows] f32
        rows = rows_hi - rows_lo
        out16 = np.ascontiguousarray(p[:, :NH]).reshape(128, NH * rows).astype(np.float16)
        out8 = np.ascontiguousarray(p[:, NH:]).reshape(128, N_DR * rows).astype(ml_dtypes.float8_e4m3)
        return out16, out8

    xh0_16, xh0_8 = pack_both(0, 128)
    xh1_16, xh1_8 = pack_both(128, 512)
    xp16 = np.empty((NB, 128, NH * RB), np.float16)
    xp8 = np.empty((NB, 128, N_DR * RB), ml_dtypes.float8_e4m3)
    for b in range(NB):
        a, c = pack_both(512 + b * RB, 512 + (b + 1) * RB)
        xp16[b], xp8[b] = a, c

    b16 = (bias.astype(np.float64) / gamma).astype(np.float16)
    in_maps = []
    for c in range(N_CORES):
        sh = s[c * F:(c + 1) * F].astype(ml_dtypes.float8_e4m3)  # [F, D_IN]
        wqq = np.stack([
            _pack(sh[q * FT:(q + 1) * FT, :]).reshape(128, KC * FT)
            for q in range(NF)
        ])
        m = {
            "wq": wqq,
            "bias": np.ascontiguousarray(b16[c * F:(c + 1) * F]).reshape(1, F),
        }
        if NH:
            m.update(xh0_16=xh0_16, xh1_16=xh1_16, xp16=xp16)
        if N_DR:
            m.update(xh0_8=xh0_8, xh1_8=xh1_8, xp8=xp8)
        in_maps.append(m)
    return in_maps


def _assemble(results):
    out = np.concatenate([results[c]["out"] for c in range(N_CORES)], axis=1)
    out *= _GAMMA[0]
    return out.reshape(B, S, D_OUT)


def kernel(x, weight, bias):
    import time
    os.environ.setdefault("BASS_NEVER_TRACE", "1")
    from concourse.bass_utils import run_bass_kernel_spmd

    in_maps = _prepare_in_maps(x, weight, bias)
    if "nc" not in _CACHE:
        _CACHE["nc"] = _build_nc()
    last_err = None
    for attempt in range(3):
        try:
            res = run_bass_kernel_spmd(
                _CACHE["nc"], in_maps, core_ids=list(range(N_CORES)))
            return _assemble(res.results)
        except Exception as e:  # transient device errors clear after ~30s
            last_err = e
            time.sleep(30 * (attempt + 1))
    raise last_err
